# revision 3
# baseline (speedup 1.0000x reference)
"""Fused attention kernel (nn_Attention_18708877541532) for 8 Trainium2 cores.

Strategy: data-parallel over batch B=16 -> 2 batches per core. Everything on
one core is computed in a "transposed" layout so no on-device transposes are
needed:
  - host passes x^T / lab^T (feature-major) in bf16
  - qT/kT = W^T @ x^T via PE  (inner on partitions)
  - scoresT[k, q] = kT_h^T-slice.T @ qT_h  (keys on partitions)
  - exp via ACT with per-key bias (fused tanh-bias + mask) and 1/sqrt(d) scale
  - attendedT[d, q] = [v | 1]^T-chunks @ expT  -> row 64 = softmax sums
  - normalize with DVE mult by partition-broadcast reciprocal sums
  - O-projection consumes attendedT directly as lhsT; + bias + residual.
"""
import numpy as np
import ml_dtypes
from contextlib import ExitStack

import concourse.bass as bass
import concourse.tile as tile
from concourse import bacc, mybir
from concourse import bass_utils

B, QL, KL = 16, 1024, 512
EMBED, HEADS, DHEAD = 768, 12, 64
INNER = HEADS * DHEAD
NCORES = 8
BLOC = B // NCORES            # 2 batches per core
P = 128
EC = EMBED // P               # 6 embed chunks
MC = INNER // P               # 6 inner chunks
KC = KL // P                  # 4 key chunks
QH = 2                        # q halves
QW = QL // QH                 # 512
QT = QW // P                  # 4 q tiles per half
SCALE = float(DHEAD) ** -0.5

F32 = mybir.dt.float32
BF16 = mybir.dt.bfloat16
BF = ml_dtypes.bfloat16

_CACHE: dict = {}


def _build():
    nc = bacc.Bacc("TRN2", target_bir_lowering=False, debug=False,
                   enable_asserts=True, num_devices=NCORES)

    xT_d = nc.dram_tensor("xT", [BLOC, EMBED, QL], BF16, kind="ExternalInput").ap()
    labT_d = nc.dram_tensor("labT", [BLOC, EMBED, KL], BF16, kind="ExternalInput").ap()
    x_d = nc.dram_tensor("x", [BLOC, QL, EMBED], F32, kind="ExternalInput").ap()
    wq_d = nc.dram_tensor("Wq", [EMBED, INNER], BF16, kind="ExternalInput").ap()
    wk_d = nc.dram_tensor("Wk", [EMBED, INNER], BF16, kind="ExternalInput").ap()
    wv_d = nc.dram_tensor("Wv", [EMBED, INNER], BF16, kind="ExternalInput").ap()
    wo_d = nc.dram_tensor("Wo", [INNER, EMBED], BF16, kind="ExternalInput").ap()
    biask_d = nc.dram_tensor("biasK", [BLOC, KL], F32, kind="ExternalInput").ap()
    bo_d = nc.dram_tensor("bo", [EMBED], F32, kind="ExternalInput").ap()
    out_d = nc.dram_tensor("out", [BLOC, QL, EMBED], F32, kind="ExternalOutput").ap()

    with tile.TileContext(nc) as tc, ExitStack() as ctx:
        sb = ctx.enter_context(tc.tile_pool(name="sb", bufs=1))
        xtp = ctx.enter_context(tc.tile_pool(name="xtp", bufs=1))
        ltp = ctx.enter_context(tc.tile_pool(name="ltp", bufs=1))
        qtp = ctx.enter_context(tc.tile_pool(name="qtp", bufs=3))
        ktp = ctx.enter_context(tc.tile_pool(name="ktp", bufs=2))
        vtp = ctx.enter_context(tc.tile_pool(name="vtp", bufs=2))
        expp = ctx.enter_context(tc.tile_pool(name="expp", bufs=2))
        attp = ctx.enter_context(tc.tile_pool(name="attp", bufs=2))
        stp = ctx.enter_context(tc.tile_pool(name="stp", bufs=2))
        smp = ctx.enter_context(tc.tile_pool(name="smp", bufs=2))
        rcp = ctx.enter_context(tc.tile_pool(name="rcp", bufs=2))
        rsp = ctx.enter_context(tc.tile_pool(name="rsp", bufs=4))
        bcp = ctx.enter_context(tc.tile_pool(name="bcp", bufs=2))
        onp = ctx.enter_context(tc.tile_pool(name="onp", bufs=2))
        oup = ctx.enter_context(tc.tile_pool(name="oup", bufs=2))
        pp = ctx.enter_context(tc.tile_pool(name="pp", bufs=3, space="PSUM"))
        ps = ctx.enter_context(tc.tile_pool(name="ps", bufs=3, space="PSUM"))
        pa = ctx.enter_context(tc.tile_pool(name="pa", bufs=2, space="PSUM"))

        # ---- persistent tiles ----
        # all four weight matrices packed: [Wq | Wk | Wv | Wo] along dim 1
        W = sb.tile([P, 4 * EC, INNER], BF16, tag="wall")
        nc.sync.dma_start(W[:, 0:EC, :], wq_d.rearrange("(c p) i -> p c i", p=P))
        nc.sync.dma_start(W[:, EC:2 * EC, :], wk_d.rearrange("(c p) i -> p c i", p=P))
        nc.sync.dma_start(W[:, 2 * EC:3 * EC, :], wv_d.rearrange("(c p) i -> p c i", p=P))
        nc.sync.dma_start(W[:, 3 * EC:4 * EC, :], wo_d.rearrange("(c p) i -> p c i", p=P))

        biask_sb = sb.tile([P, BLOC, KC], F32, tag="biask")
        for b in range(BLOC):
            nc.sync.dma_start(biask_sb[:, b, :],
                              biask_d[b].rearrange("(c p) -> p c", p=P))
        bo_bc = sb.tile([P, EMBED], F32, tag="bob")
        nc.sync.dma_start(bo_bc[:], bo_d[None, :].to_broadcast((P, EMBED)))

        xT_sb: dict = {}
        labT_sb: dict = {}
        qT_sb: dict = {}
        kT_sb: dict = {}
        v_sb: dict = {}
        att_sb: dict = {}

        def g_qkv(b, sections):
            if "init" in sections:
                xt = xtp.tile([P, EC, QL], BF16, tag="xT")
                nc.sync.dma_start(xt[:], xT_d[b].rearrange("(c p) t -> p c t", p=P))
                xT_sb[b] = xt
                lt = ltp.tile([P, EC, KL], BF16, tag="labT")
                nc.sync.dma_start(lt[:], labT_d[b].rearrange("(c p) t -> p c t", p=P))
                labT_sb[b] = lt
                yield
            if "q0" in sections or "q1" in sections:
                for qh in ([0] if "q0" in sections else []) + ([1] if "q1" in sections else []):
                    qt_t = qtp.tile([P, MC, QW], BF16, tag="qT")
                    qT_sb[(b, qh)] = qt_t
                    for m in range(MC):
                        pt = pp.tile([P, 512], F32, tag="pp")
                        for c in range(EC):
                            nc.tensor.matmul(
                                pt[:], W[:, c, m * P:(m + 1) * P],
                                xT_sb[b][:, c, qh * QW:(qh + 1) * QW],
                                start=(c == 0), stop=(c == EC - 1))
                        nc.vector.tensor_copy(qt_t[:, m, :], pt[:])
                        yield
            if "k" in sections:
                kt_t = ktp.tile([P, MC, KL], BF16, tag="kT")
                kT_sb[b] = kt_t
                for m in range(MC):
                    pt = pp.tile([P, 512], F32, tag="pp")
                    for c in range(EC):
                        nc.tensor.matmul(
                            pt[:], W[:, EC + c, m * P:(m + 1) * P],
                            labT_sb[b][:, c, :],
                            start=(c == 0), stop=(c == EC - 1))
                    nc.vector.tensor_copy(kt_t[:, m, :], pt[:])
                    yield
            if "v" in sections:
                v_t = vtp.tile([P, KC, HEADS, DHEAD + 1], BF16, tag="v")
                v_sb[b] = v_t
                nc.vector.memset(v_t[:, :, :, DHEAD:DHEAD + 1], 1.0)
                for t in range(KC):
                    for n0, nw in ((0, 512), (512, 256)):
                        pt = pp.tile([P, 512], F32, tag="pp")
                        for c in range(EC):
                            nc.tensor.matmul(
                                pt[:, :nw], labT_sb[b][:, c, t * P:(t + 1) * P],
                                W[:, 2 * EC + c, n0:n0 + nw],
                                start=(c == 0), stop=(c == EC - 1))
                        for h in range(n0 // DHEAD, (n0 + nw) // DHEAD):
                            nc.vector.tensor_copy(
                                v_t[:, t, h, 0:DHEAD],
                                pt[:, h * DHEAD - n0:h * DHEAD - n0 + DHEAD])
                        yield

        def g_att(b, qh):
            att_t = attp.tile([P, MC, QW], BF16, tag="att")
            att_sb[(b, qh)] = att_t
            sums_t = smp.tile([HEADS, QW], BF16, tag="sums")
            qt_t = qT_sb[(b, qh)]
            kt_t = kT_sb[b]
            v_t = v_sb[b]
            for h in range(HEADS):
                p0 = (h % 2) * DHEAD
                hc = h // 2
                ex_t = expp.tile([P, KC, QW], BF16, tag="exp")
                for kc in range(KC):
                    ss = ps.tile([P, QW], F32, tag="ps")
                    nc.tensor.matmul(ss[:],
                                     kt_t[p0:p0 + DHEAD, hc, kc * P:(kc + 1) * P],
                                     qt_t[p0:p0 + DHEAD, hc, :])
                    nc.scalar.activation(ex_t[:, kc, :], ss[:],
                                         mybir.ActivationFunctionType.Exp,
                                         bias=biask_sb[:, b, kc:kc + 1], scale=SCALE)
                pa_t = pa.tile([DHEAD + 1, QW], F32, tag="pa")
                for kc in range(KC):
                    nc.tensor.matmul(pa_t[:], v_t[:, kc, h, :], ex_t[:, kc, :],
                                     start=(kc == 0), stop=(kc == KC - 1))
                st_t = stp.tile([DHEAD + 1, QW], BF16, tag="stage")
                nc.vector.tensor_copy(st_t[:], pa_t[:])
                nc.sync.dma_start(att_t[p0:p0 + DHEAD, hc, :], st_t[0:DHEAD, :])
                nc.sync.dma_start(sums_t[h:h + 1, :], st_t[DHEAD:DHEAD + 1, :])
                yield
            rec_t = rcp.tile([HEADS, QW], F32, tag="rec")
            nc.vector.reciprocal(rec_t[:], sums_t[:])
            yield
            for hc in range(MC):
                ra = rsp.tile([1, QW], F32, tag="rstage")
                nc.sync.dma_start(ra[:], rec_t[2 * hc:2 * hc + 1, :])
                rb = rsp.tile([1, QW], F32, tag="rstage")
                nc.sync.dma_start(rb[:], rec_t[2 * hc + 1:2 * hc + 2, :])
                ba = bcp.tile([P, QW], F32, tag="bc")
                nc.gpsimd.partition_broadcast(ba[0:DHEAD, :], ra[:])
                bb = bcp.tile([P, QW], F32, tag="bc")
                nc.gpsimd.partition_broadcast(bb[:], rb[:])
                nc.vector.tensor_mul(att_t[0:DHEAD, hc, :],
                                     att_t[0:DHEAD, hc, :], ba[0:DHEAD, :])
                nc.vector.tensor_mul(att_t[DHEAD:P, hc, :],
                                     att_t[DHEAD:P, hc, :], bb[DHEAD:P, :])
                yield

        def g_out(b, qh):
            att_t = att_sb[(b, qh)]
            for qt in range(QT):
                qg = qh * QT + qt
                xn = onp.tile([P, EMBED], F32, tag="xn")
                nc.sync.dma_start(xn[:], x_d[b, qg * P:(qg + 1) * P, :])
                ou = oup.tile([P, EMBED], F32, tag="ou")
                for n0, nw in ((0, 512), (512, 256)):
                    po = pp.tile([P, 512], F32, tag="pp")
                    for c in range(MC):
                        nc.tensor.matmul(po[:, :nw],
                                         att_t[:, c, qt * P:(qt + 1) * P],
                                         W[:, 3 * EC + c, n0:n0 + nw],
                                         start=(c == 0), stop=(c == MC - 1))
                    nc.vector.tensor_add(ou[:, n0:n0 + nw], po[:, :nw],
                                         bo_bc[:, n0:n0 + nw])
                    nc.vector.tensor_add(ou[:, n0:n0 + nw], ou[:, n0:n0 + nw],
                                         xn[:, n0:n0 + nw])
                nc.sync.dma_start(out_d[b, qg * P:(qg + 1) * P, :], ou[:])
                yield

        def rr(*gens):
            live = [iter(g) for g in gens]
            while live:
                for g in list(live):
                    try:
                        next(g)
                    except StopIteration:
                        live.remove(g)

        rr(g_qkv(0, ("init", "q0", "q1", "k", "v")))
        rr(g_att(0, 0), g_qkv(1, ("init", "q0", "k")))
        rr(g_att(0, 1), g_qkv(1, ("v", "q1")), g_out(0, 0))
        rr(g_att(1, 0), g_out(0, 1))
        rr(g_att(1, 1), g_out(1, 0))
        rr(g_out(1, 1))

    nc.compile()
    return nc


def _get_nc():
    if "nc" not in _CACHE:
        _CACHE["nc"] = _build()
    return _CACHE["nc"]


def _prep(inputs):
    x = np.asarray(inputs["image_embeddings"], dtype=np.float32)
    lab = np.asarray(inputs["lab_embeddings"], dtype=np.float32)
    lv = np.asarray(inputs["lab_values"], dtype=np.float32)
    Wq = np.asarray(inputs["Wq"], dtype=np.float32)
    Wk = np.asarray(inputs["Wk"], dtype=np.float32)
    Wv = np.asarray(inputs["Wv"], dtype=np.float32)
    Wo = np.asarray(inputs["Wo"], dtype=np.float32)
    bo = np.asarray(inputs["bo"], dtype=np.float32)
    table = np.asarray(inputs["bias_table"], dtype=np.float32)
    vp_w = np.asarray(inputs["vp_w"], dtype=np.float32)
    vp_b = np.asarray(inputs["vp_b"], dtype=np.float32)
    fus_w = np.asarray(inputs["fus_w"], dtype=np.float32)
    fus_b = np.asarray(inputs["fus_b"], dtype=np.float32)
    idx = np.asarray(inputs["lab_test_indices"])
    mask = np.asarray(inputs["mask"])

    # per-key additive bias: embedding + linear + tanh + clamp, then mask
    tb = table[idx, 0]                                   # [B, KL] f32
    vb = lv * vp_w[0, 0] + vp_b[0]
    tv = np.tanh(tb * fus_w[0, 0] + vb * fus_w[1, 0] + fus_b[0])
    tv = np.clip(tv, -5.0, 5.0).astype(np.float32)
    biasK = np.where(mask == 0, np.float32(-1e9), tv).astype(np.float32)

    xT = np.ascontiguousarray(x.transpose(0, 2, 1)).astype(BF)
    labT = np.ascontiguousarray(lab.transpose(0, 2, 1)).astype(BF)
    shared = {
        "Wq": Wq.astype(BF), "Wk": Wk.astype(BF), "Wv": Wv.astype(BF),
        "Wo": Wo.astype(BF), "bo": bo,
    }
    in_maps = []
    for i in range(NCORES):
        s = slice(BLOC * i, BLOC * (i + 1))
        in_maps.append({
            "xT": xT[s], "labT": labT[s],
            "x": np.ascontiguousarray(x[s]),
            "biasK": np.ascontiguousarray(biasK[s]),
            **shared,
        })
    return in_maps


def run(inputs, trace=False, tmpdir=None):
    nc = _get_nc()
    in_maps = _prep(inputs)
    res = bass_utils.run_bass_kernel_spmd(
        nc, in_maps, core_ids=list(range(NCORES)), trace=trace, tmpdir=tmpdir)
    out = np.concatenate([res.results[i]["out"] for i in range(NCORES)], axis=0)
    return out, res


def kernel(**inputs) -> np.ndarray:
    out, _ = run(inputs)
    return out


if __name__ == "__main__":
    rng = np.random.default_rng(0)
    fake = {
        "image_embeddings": rng.standard_normal((B, QL, EMBED)).astype(np.float32),
        "lab_embeddings": rng.standard_normal((B, KL, EMBED)).astype(np.float32),
        "lab_values": rng.standard_normal((B, KL)).astype(np.float32),
        "Wq": (rng.standard_normal((EMBED, INNER)) * 0.02).astype(np.float32),
        "Wk": (rng.standard_normal((EMBED, INNER)) * 0.02).astype(np.float32),
        "Wv": (rng.standard_normal((EMBED, INNER)) * 0.02).astype(np.float32),
        "Wo": (rng.standard_normal((INNER, EMBED)) * 0.02).astype(np.float32),
        "bo": np.zeros(EMBED, np.float32),
        "bias_table": (rng.standard_normal((1001, 1)) * 0.02).astype(np.float32),
        "vp_w": rng.standard_normal((1, 1)).astype(np.float32),
        "vp_b": np.zeros(1, np.float32),
        "fus_w": rng.standard_normal((2, 1)).astype(np.float32),
        "fus_b": np.zeros(1, np.float32),
        "lab_test_indices": rng.integers(0, 1001, (B, KL)),
        "mask": rng.integers(0, 2, (B, KL)).astype(np.int32),
    }
    out = kernel(**fake)
    print("out", out.shape, out.dtype, float(np.abs(out).max()))


# revision 6
# speedup vs baseline: 1.0958x; 1.0958x over previous
"""Fused attention kernel (nn_Attention_18708877541532) for 8 Trainium2 cores.

Strategy: data-parallel over batch B=16 -> 2 batches per core. Everything on
one core is computed in a "transposed" layout so no on-device transposes are
needed:
  - host passes x^T / lab^T (feature-major) in bf16
  - qT/kT = W^T @ x^T via PE  (inner on partitions)
  - scoresT[k, q] = kT_h-slice.T @ qT_h  (keys on partitions, both q-halves
    into one 2-bank PSUM tile)
  - exp via one ACT op per (head, key-chunk) with per-key bias (fused
    tanh-bias + mask) and 1/sqrt(d) scale
  - attendedT[d, q] = [v | 1]-chunks.T @ expT  -> row 64 = softmax sums
  - per-head-pair normalization with fast reciprocal + gpsimd partition
    broadcast (no end-of-batch sync)
  - O-projection consumes attendedT directly as lhsT; + bias + residual.
"""
import numpy as np
import ml_dtypes
from contextlib import ExitStack

import concourse.bass as bass
import concourse.tile as tile
from concourse import bacc, mybir
from concourse import bass_utils

B, QL, KL = 16, 1024, 512
EMBED, HEADS, DHEAD = 768, 12, 64
INNER = HEADS * DHEAD
NCORES = 8
BLOC = B // NCORES            # 2 batches per core
P = 128
EC = EMBED // P               # 6 embed chunks
MC = INNER // P               # 6 inner chunks
KC = KL // P                  # 4 key chunks
QH = 2                        # q halves
QW = QL // QH                 # 512
QT = QW // P                  # 4 q tiles per half
SCALE = float(DHEAD) ** -0.5

F32 = mybir.dt.float32
BF16 = mybir.dt.bfloat16
BF = ml_dtypes.bfloat16

_CACHE: dict = {}


def _build():
    nc = bacc.Bacc("TRN2", target_bir_lowering=False, debug=False,
                   enable_asserts=True, num_devices=NCORES)

    xT_d = nc.dram_tensor("xT", [BLOC, EMBED, QL], BF16, kind="ExternalInput").ap()
    labT_d = nc.dram_tensor("labT", [BLOC, EMBED, KL], BF16, kind="ExternalInput").ap()
    x_d = nc.dram_tensor("x", [BLOC, QL, EMBED], F32, kind="ExternalInput").ap()
    wq_d = nc.dram_tensor("Wq", [EMBED, INNER], BF16, kind="ExternalInput").ap()
    wk_d = nc.dram_tensor("Wk", [EMBED, INNER], BF16, kind="ExternalInput").ap()
    wv_d = nc.dram_tensor("Wv", [EMBED, INNER], BF16, kind="ExternalInput").ap()
    wo_d = nc.dram_tensor("Wo", [INNER, EMBED], BF16, kind="ExternalInput").ap()
    biask_d = nc.dram_tensor("biasK", [BLOC, KL], F32, kind="ExternalInput").ap()
    out_d = nc.dram_tensor("out", [BLOC, QL, EMBED], F32, kind="ExternalOutput").ap()

    with tile.TileContext(nc) as tc, ExitStack() as ctx:
        sb = ctx.enter_context(tc.tile_pool(name="sb", bufs=1))
        xtp = ctx.enter_context(tc.tile_pool(name="xtp", bufs=1))
        ltp = ctx.enter_context(tc.tile_pool(name="ltp", bufs=1))
        qtp = ctx.enter_context(tc.tile_pool(name="qtp", bufs=2))
        ktp = ctx.enter_context(tc.tile_pool(name="ktp", bufs=2))
        vtp = ctx.enter_context(tc.tile_pool(name="vtp", bufs=2))
        expp = ctx.enter_context(tc.tile_pool(name="expp", bufs=2))
        attp = ctx.enter_context(tc.tile_pool(name="attp", bufs=2))
        stp = ctx.enter_context(tc.tile_pool(name="stp", bufs=3))
        smp = ctx.enter_context(tc.tile_pool(name="smp", bufs=2))
        rcp = ctx.enter_context(tc.tile_pool(name="rcp", bufs=2))
        rsp = ctx.enter_context(tc.tile_pool(name="rsp", bufs=4))
        bcp = ctx.enter_context(tc.tile_pool(name="bcp", bufs=3))
        onp = ctx.enter_context(tc.tile_pool(name="onp", bufs=2))
        oup = ctx.enter_context(tc.tile_pool(name="oup", bufs=2))
        pp = ctx.enter_context(tc.tile_pool(name="pp", bufs=2, space="PSUM"))
        ps = ctx.enter_context(tc.tile_pool(name="ps", bufs=2, space="PSUM"))
        pa = ctx.enter_context(tc.tile_pool(name="pa", bufs=2, space="PSUM"))

        # ---- persistent tiles; chunked DMAs so compute can start early ----
        W = sb.tile([P, 4 * EC, INNER], BF16, tag="wall")
        wq_r = wq_d.rearrange("(c p) i -> p c i", p=P)
        wk_r = wk_d.rearrange("(c p) i -> p c i", p=P)
        wv_r = wv_d.rearrange("(c p) i -> p c i", p=P)
        wo_r = wo_d.rearrange("(c p) i -> p c i", p=P)

        biask_sb = sb.tile([P, BLOC, KC], F32, tag="biask")

        def g_preload():
            for c in range(EC):
                nc.sync.dma_start(W[:, c, :], wq_r[:, c, :])
            for b in range(BLOC):
                nc.sync.dma_start(biask_sb[:, b, :],
                                  biask_d[b].rearrange("(c p) -> p c", p=P))
            yield
            for c in range(EC):
                nc.sync.dma_start(W[:, EC + c, :], wk_r[:, c, :])
            yield
            for c in range(EC):
                nc.sync.dma_start(W[:, 2 * EC + c, :], wv_r[:, c, :])
            yield
            for c in range(EC):
                nc.sync.dma_start(W[:, 3 * EC + c, :], wo_r[:, c, :])
            yield

        xT_sb: dict = {}
        labT_sb: dict = {}
        qT_sb: dict = {}
        kT_sb: dict = {}
        v_sb: dict = {}
        att_sb: dict = {}

        def g_qkv(b, sections):
            if "init" in sections:
                xt = xtp.tile([P, EC, QL], BF16, tag="xT")
                xr = xT_d[b].rearrange("(c p) t -> p c t", p=P)
                for c in range(EC):
                    nc.sync.dma_start(xt[:, c, :], xr[:, c, :])
                xT_sb[b] = xt
                lt = ltp.tile([P, EC, KL], BF16, tag="labT")
                lr = labT_d[b].rearrange("(c p) t -> p c t", p=P)
                for c in range(EC):
                    nc.sync.dma_start(lt[:, c, :], lr[:, c, :])
                labT_sb[b] = lt
                yield
            if "q" in sections:
                qt_t = qtp.tile([P, MC, QL], BF16, tag="qT")
                qT_sb[b] = qt_t
                for m in range(MC):
                    for qh in range(QH):
                        pt = pp.tile([P, 512], F32, tag="pp")
                        for c in range(EC):
                            nc.tensor.matmul(
                                pt[:], W[:, c, m * P:(m + 1) * P],
                                xT_sb[b][:, c, qh * QW:(qh + 1) * QW],
                                start=(c == 0), stop=(c == EC - 1))
                        nc.vector.tensor_copy(qt_t[:, m, qh * QW:(qh + 1) * QW], pt[:])
                        yield
            if "k" in sections:
                kt_t = ktp.tile([P, MC, KL], BF16, tag="kT")
                kT_sb[b] = kt_t
                for m in range(MC):
                    pt = pp.tile([P, 512], F32, tag="pp")
                    for c in range(EC):
                        nc.tensor.matmul(
                            pt[:], W[:, EC + c, m * P:(m + 1) * P],
                            labT_sb[b][:, c, :],
                            start=(c == 0), stop=(c == EC - 1))
                    nc.vector.tensor_copy(kt_t[:, m, :], pt[:])
                    yield
            if "v" in sections:
                v_t = vtp.tile([P, KC, HEADS, DHEAD + 1], BF16, tag="v")
                v_sb[b] = v_t
                nc.vector.memset(v_t[:, :, :, DHEAD:DHEAD + 1], 1.0)
                for t in range(KC):
                    for n0, nw in ((0, 512), (512, 256)):
                        pt = pp.tile([P, 512], F32, tag="pp")
                        for c in range(EC):
                            nc.tensor.matmul(
                                pt[:, :nw], labT_sb[b][:, c, t * P:(t + 1) * P],
                                W[:, 2 * EC + c, n0:n0 + nw],
                                start=(c == 0), stop=(c == EC - 1))
                        h0, h1 = n0 // DHEAD, (n0 + nw) // DHEAD
                        nc.vector.tensor_copy(
                            v_t[:, t, h0:h1, 0:DHEAD],
                            pt[:, :nw].rearrange("p (h d) -> p h d", d=DHEAD))
                        yield

        def g_att(b):
            att_t = {qh: attp.tile([P, MC, QW], BF16, tag="att", name=f"att_{b}_{qh}")
                     for qh in range(QH)}
            for qh in range(QH):
                att_sb[(b, qh)] = att_t[qh]
            qt_t = qT_sb[b]
            kt_t = kT_sb[b]
            v_t = v_sb[b]
            pairsums = None
            for h in range(HEADS):
                par = h % 2
                p0 = par * DHEAD
                hc = h // 2
                if par == 0:
                    pairsums = smp.tile([2, QH, QW], F32, tag="sums")
                ex_t = expp.tile([P, KC, QL], BF16, tag="exp")
                for kc in range(KC):
                    ss = ps.tile([P, QL], F32, tag="ps")
                    for qh in range(QH):
                        nc.tensor.matmul(
                            ss[:, qh * QW:(qh + 1) * QW],
                            kt_t[p0:p0 + DHEAD, hc, kc * P:(kc + 1) * P],
                            qt_t[p0:p0 + DHEAD, hc, qh * QW:(qh + 1) * QW])
                    nc.scalar.activation(ex_t[:, kc, :], ss[:],
                                         mybir.ActivationFunctionType.Exp,
                                         bias=biask_sb[:, b, kc:kc + 1], scale=SCALE)
                for qh in range(QH):
                    pa_t = pa.tile([DHEAD + 1, QW], F32, tag="pa")
                    for kc in range(KC):
                        nc.tensor.matmul(pa_t[:], v_t[:, kc, h, :],
                                         ex_t[:, kc, qh * QW:(qh + 1) * QW],
                                         start=(kc == 0), stop=(kc == KC - 1))
                    st_t = stp.tile([DHEAD + 1, QW], F32, tag="stage")
                    nc.vector.tensor_copy(st_t[:], pa_t[:])
                    nc.gpsimd.dma_start(att_t[qh][p0:p0 + DHEAD, hc, :],
                                        st_t[0:DHEAD, :])
                    nc.sync.dma_start(pairsums[par:par + 1, qh, :],
                                      st_t[DHEAD:DHEAD + 1, :])
                yield
                if par == 1:
                    # normalize this head pair for both q-halves
                    rec2 = rcp.tile([2, QH, QW], F32, tag="rec")
                    nc.vector.reciprocal_approx_fast(rec2[:], pairsums[:])
                    for qh in range(QH):
                        ra = rsp.tile([1, QW], F32, tag="rstage")
                        nc.sync.dma_start(ra[:], rec2[0:1, qh, :])
                        rb = rsp.tile([1, QW], F32, tag="rstage")
                        nc.sync.dma_start(rb[:], rec2[1:2, qh, :])
                        ba = bcp.tile([P, QW], F32, tag="bc")
                        nc.gpsimd.partition_broadcast(ba[0:DHEAD, :], ra[:])
                        bb = bcp.tile([P, QW], F32, tag="bc")
                        nc.gpsimd.partition_broadcast(bb[:], rb[:])
                        a_t = att_t[qh]
                        nc.vector.tensor_mul(a_t[0:DHEAD, hc, :],
                                             a_t[0:DHEAD, hc, :], ba[0:DHEAD, :])
                        nc.vector.tensor_mul(a_t[DHEAD:P, hc, :],
                                             a_t[DHEAD:P, hc, :], bb[DHEAD:P, :])
                    yield

        def g_out(b, qh):
            att_t = att_sb[(b, qh)]
            for qt in range(QT):
                qg = qh * QT + qt
                xn = onp.tile([P, EMBED], F32, tag="xn")
                nc.sync.dma_start(xn[:], x_d[b, qg * P:(qg + 1) * P, :])
                ou = oup.tile([P, EMBED], F32, tag="ou")
                for n0, nw in ((0, 512), (512, 256)):
                    po = pp.tile([P, 512], F32, tag="pp")
                    for c in range(MC):
                        nc.tensor.matmul(po[:, :nw],
                                         att_t[:, c, qt * P:(qt + 1) * P],
                                         W[:, 3 * EC + c, n0:n0 + nw],
                                         start=(c == 0), stop=(c == MC - 1))
                    nc.vector.tensor_add(ou[:, n0:n0 + nw], po[:, :nw],
                                         xn[:, n0:n0 + nw])
                nc.sync.dma_start(out_d[b, qg * P:(qg + 1) * P, :], ou[:])
                yield

        def rr(*gens):
            live = [iter(g) for g in gens]
            while live:
                for g in list(live):
                    try:
                        next(g)
                    except StopIteration:
                        live.remove(g)

        rr(g_preload(), g_qkv(0, ("init", "q", "k")))
        rr(g_qkv(0, ("v",)))
        rr(g_att(0), g_qkv(1, ("init", "q", "k", "v")))
        rr(g_att(1), g_out(0, 0), g_out(0, 1))
        rr(g_out(1, 0), g_out(1, 1))

    nc.compile()
    return nc


def _get_nc():
    if "nc" not in _CACHE:
        _CACHE["nc"] = _build()
    return _CACHE["nc"]


def _prep(inputs):
    x = np.asarray(inputs["image_embeddings"], dtype=np.float32)
    lab = np.asarray(inputs["lab_embeddings"], dtype=np.float32)
    lv = np.asarray(inputs["lab_values"], dtype=np.float32)
    Wq = np.asarray(inputs["Wq"], dtype=np.float32)
    Wk = np.asarray(inputs["Wk"], dtype=np.float32)
    Wv = np.asarray(inputs["Wv"], dtype=np.float32)
    Wo = np.asarray(inputs["Wo"], dtype=np.float32)
    bo = np.asarray(inputs["bo"], dtype=np.float32)
    table = np.asarray(inputs["bias_table"], dtype=np.float32)
    vp_w = np.asarray(inputs["vp_w"], dtype=np.float32)
    vp_b = np.asarray(inputs["vp_b"], dtype=np.float32)
    fus_w = np.asarray(inputs["fus_w"], dtype=np.float32)
    fus_b = np.asarray(inputs["fus_b"], dtype=np.float32)
    idx = np.asarray(inputs["lab_test_indices"])
    mask = np.asarray(inputs["mask"])

    # per-key additive bias: embedding + linear + tanh + clamp, then mask
    tb = table[idx, 0]                                   # [B, KL] f32
    vb = lv * vp_w[0, 0] + vp_b[0]
    tv = np.tanh(tb * fus_w[0, 0] + vb * fus_w[1, 0] + fus_b[0])
    tv = np.clip(tv, -5.0, 5.0).astype(np.float32)
    biasK = np.where(mask == 0, np.float32(-1e9), tv).astype(np.float32)

    xT = np.ascontiguousarray(x.transpose(0, 2, 1)).astype(BF)
    labT = np.ascontiguousarray(lab.transpose(0, 2, 1)).astype(BF)
    x_pb = x + bo  # fold output bias into the residual
    shared = {
        "Wq": Wq.astype(BF), "Wk": Wk.astype(BF), "Wv": Wv.astype(BF),
        "Wo": Wo.astype(BF),
    }
    in_maps = []
    for i in range(NCORES):
        s = slice(BLOC * i, BLOC * (i + 1))
        in_maps.append({
            "xT": xT[s], "labT": labT[s],
            "x": np.ascontiguousarray(x_pb[s]),
            "biasK": np.ascontiguousarray(biasK[s]),
            **shared,
        })
    return in_maps


def run(inputs, trace=False, tmpdir=None):
    nc = _get_nc()
    in_maps = _prep(inputs)
    res = bass_utils.run_bass_kernel_spmd(
        nc, in_maps, core_ids=list(range(NCORES)), trace=trace, tmpdir=tmpdir)
    out = np.concatenate([res.results[i]["out"] for i in range(NCORES)], axis=0)
    return out, res


def kernel(**inputs) -> np.ndarray:
    out, _ = run(inputs)
    return out


if __name__ == "__main__":
    rng = np.random.default_rng(0)
    fake = {
        "image_embeddings": rng.standard_normal((B, QL, EMBED)).astype(np.float32),
        "lab_embeddings": rng.standard_normal((B, KL, EMBED)).astype(np.float32),
        "lab_values": rng.standard_normal((B, KL)).astype(np.float32),
        "Wq": (rng.standard_normal((EMBED, INNER)) * 0.02).astype(np.float32),
        "Wk": (rng.standard_normal((EMBED, INNER)) * 0.02).astype(np.float32),
        "Wv": (rng.standard_normal((EMBED, INNER)) * 0.02).astype(np.float32),
        "Wo": (rng.standard_normal((INNER, EMBED)) * 0.02).astype(np.float32),
        "bo": np.zeros(EMBED, np.float32),
        "bias_table": (rng.standard_normal((1001, 1)) * 0.02).astype(np.float32),
        "vp_w": rng.standard_normal((1, 1)).astype(np.float32),
        "vp_b": np.zeros(1, np.float32),
        "fus_w": rng.standard_normal((2, 1)).astype(np.float32),
        "fus_b": np.zeros(1, np.float32),
        "lab_test_indices": rng.integers(0, 1001, (B, KL)),
        "mask": rng.integers(0, 2, (B, KL)).astype(np.int32),
    }
    out = kernel(**fake)
    print("out", out.shape, out.dtype, float(np.abs(out).max()))
